# revision 2
# baseline (speedup 1.0000x reference)
"""Doc2vec embedding lookup + negative-sampling scores on 8 trn2 cores, v2.

reference:
    x[b, :] = D[doc_ids[b]] + sum_c W[context_ids[b, c]]      # (B, 256)
    scores[b, k] = dot(x[b], O[:, target_noise_ids[b, k]])    # (B, 6)

Strategy (per core, 512 items = 4 tiles of 128):
- Tables cast to bf16 on host. All gathers via the vectorized InstDMAGatherAnt
  ucode (int16 indices, ~7.75ns/idx desc-gen per Q7 pair), spread across all
  4 SWDGE queues so 4 Q7 core pairs generate descriptors concurrently
  (measured: queue!=0 gathers occupy the engine ~70ns, work runs async).
- int16 indices cap a table at 32768 rows, so W (50000) and O.T (50000) are
  split into lo[0:32767]/hi[32767:] windows, each with a zeros row at index 0.
  Each (item, slot) appears in BOTH windows' positional grids; the window not
  owning the id gathers the zeros row (dummy), so a single fused reduce over
  [lo slots | hi slots | doc] gives x, and lo+hi dot products sum to scores.
- doc ids (100000 > 32768) are handled by globally sorting items by doc_id:
  each core's 512 items then span ~12.5k doc rows, and a per-core 32768-row
  slice of D covers them with int16 indices.
- Per-(stream, tile) gather pieces, round-robin across queues, so tile t's
  data lands ~(t+1)/4 of the way through desc-gen and the DVE pipeline
  (1 fused reduce + 1 mult + 1 fused XY-reduce per tile) hides under it.
"""

import sys

sys.path.insert(0, "/opt/trn_rl_repo")

from contextlib import ExitStack

import ml_dtypes
import numpy as np

from concourse import bacc, bass, mybir
from concourse.bass_utils import run_bass_kernel_spmd

VEC = 256
N_DOCS = 100000
N_WORDS = 50000
B = 4096
N_CTX = 8
N_NOISE = 6
N_CORES = 8
BPC = B // N_CORES  # 512
P = 128
TILES = BPC // P  # 4
WB = 32767  # window boundary: lo ids [0, WB), hi ids [WB, N_WORDS)
WLO_R = WB + 1  # 32768 rows incl zeros row 0
WHI_R = N_WORDS - WB + 1  # 17234 rows incl zeros row 0
DTAB_R = 32768  # per-core doc table slice
# per-tile slot layout in G: [ctx lo 0..7 | ctx hi 8..15 | doc 16]
G_SLOTS = 2 * N_CTX + 1  # 17
N_SLOTS = 2 * N_NOISE  # 12: [noise lo 0..5 | noise hi 6..11]

# gather pieces: (name, stream, tile, n_idx). stream decides src table and
# dst slot range; per-queue lists are ordered tile-0-first for pipelining.
# queue assignment balances desc counts: q0=3840 q1=3840 q2=3456 q3=3712.
PIECES = {
    1: [("ctxA", 0), ("ctxA", 1), ("ctxA", 2), ("ctxA", 3)],
    2: [("ctxB", 0), ("ctxB", 1), ("ctxB", 2), ("ctxB", 3)],
    3: [
        ("doc", 0),
        ("noiB", 0),
        ("doc", 1),
        ("noiB", 1),
        ("doc", 2),
        ("noiB", 2),
        ("doc", 3),
        ("noiB", 3),
    ],
    0: [("noiA", 0), ("noiA", 1), ("noiA", 2), ("noiA", 3)],
}
STREAM_N = {"ctxA": P * N_CTX, "ctxB": P * N_CTX, "noiA": P * N_NOISE, "noiB": P * N_NOISE, "doc": P}
IDX_COLS = sum(n // 16 for q in PIECES.values() for (s, t) in q for n in [STREAM_N[s]])

_nc_cache = None


def _piece_order():
    """Per tile-round: async queues 1,2,3 first, blocking queue 0 last."""
    order = []
    for t in range(TILES):
        order.append((1, PIECES[1][t]))
        order.append((2, PIECES[2][t]))
        order.append((3, PIECES[3][2 * t]))
        order.append((3, PIECES[3][2 * t + 1]))
        order.append((0, PIECES[0][t]))
    return order


def _idx_layout():
    """Assign idx-tensor column ranges to pieces in emission order."""
    lay = {}
    c = 0
    for q, (s, t) in _piece_order():
        n = STREAM_N[s]
        lay[(s, t)] = (c, n)
        c += n // 16
    assert c == IDX_COLS
    return lay


def build_nc():
    nc = bacc.Bacc(None, target_bir_lowering=False, debug=False, num_swdge_queues=4)
    wlo = nc.declare_dram_parameter("wlo", [WLO_R, VEC], mybir.dt.bfloat16, isOutput=False)
    whi = nc.declare_dram_parameter("whi", [WHI_R, VEC], mybir.dt.bfloat16, isOutput=False)
    olo = nc.declare_dram_parameter("olo", [WLO_R, VEC], mybir.dt.bfloat16, isOutput=False)
    ohi = nc.declare_dram_parameter("ohi", [WHI_R, VEC], mybir.dt.bfloat16, isOutput=False)
    dtab = nc.declare_dram_parameter("dtab", [DTAB_R, VEC], mybir.dt.bfloat16, isOutput=False)
    idx = nc.declare_dram_parameter("idx", [P, IDX_COLS], mybir.dt.int16, isOutput=False)
    out = nc.declare_dram_parameter("out", [P, TILES * N_NOISE], mybir.dt.float32, isOutput=True)
    src_of = {"ctxA": wlo, "ctxB": whi, "noiA": olo, "noiB": ohi, "doc": dtab}
    lay = _idx_layout()

    with ExitStack() as ctx:
        block = ctx.enter_context(nc.Block(no_gpsimd_drain=True))
        s_idx = ctx.enter_context(nc.semaphore("s_idx"))
        s_tile = [ctx.enter_context(nc.semaphore(f"s_tile{t}")) for t in range(TILES)]
        s_vec = ctx.enter_context(nc.semaphore("s_vec"))
        s_out = ctx.enter_context(nc.semaphore("s_out"))

        idx_t = ctx.enter_context(nc.sbuf_tensor("idx_t", [P, IDX_COLS], mybir.dt.int16))
        G = ctx.enter_context(
            nc.sbuf_tensor("G", [P, TILES * G_SLOTS * VEC], mybir.dt.bfloat16)
        )
        NB = ctx.enter_context(
            nc.sbuf_tensor("NB", [P, TILES * N_SLOTS * VEC], mybir.dt.bfloat16)
        )
        xb = ctx.enter_context(nc.sbuf_tensor("xb", [P, TILES * VEC], mybir.dt.bfloat16))
        s8 = ctx.enter_context(nc.sbuf_tensor("s8", [P, 8 * VEC], mybir.dt.bfloat16))
        s4 = ctx.enter_context(nc.sbuf_tensor("s4", [P, 4 * VEC], mybir.dt.bfloat16))
        s2 = ctx.enter_context(nc.sbuf_tensor("s2", [P, 2 * VEC], mybir.dt.bfloat16))
        s1 = ctx.enter_context(nc.sbuf_tensor("s1", [P, VEC], mybir.dt.bfloat16))
        sc12 = ctx.enter_context(nc.sbuf_tensor("sc12", [P, N_SLOTS], mybir.dt.float32))
        prod = ctx.enter_context(
            nc.sbuf_tensor("prod", [P, N_SLOTS * VEC], mybir.dt.bfloat16)
        )
        sc = ctx.enter_context(
            nc.sbuf_tensor("sc", [P, TILES * N_NOISE], mybir.dt.float32)
        )

        def g_dst(s, t):
            base = t * G_SLOTS * VEC
            if s == "ctxA":
                return G[:, base : base + N_CTX * VEC], N_CTX
            if s == "ctxB":
                return G[:, base + N_CTX * VEC : base + 2 * N_CTX * VEC], N_CTX
            if s == "doc":
                return G[:, base + 2 * N_CTX * VEC : base + G_SLOTS * VEC], 1
            nbase = t * N_SLOTS * VEC
            if s == "noiA":
                return NB[:, nbase : nbase + N_NOISE * VEC], N_NOISE
            return NB[:, nbase + N_NOISE * VEC : nbase + N_SLOTS * VEC], N_NOISE

        @block.sync
        def _(s: bass.BassEngine):
            s.dma_start(out=idx_t[:, :], in_=idx[:, :]).then_inc(s_idx, 16)
            s.wait_ge(s_vec, TILES)
            s.dma_start(out=out[:, :], in_=sc[:, :]).then_inc(s_out, 16)
            s.wait_ge(s_out, 16)

        @block.gpsimd
        def _(g: bass.BassGpSimd):
            from concourse.library_config import mlp

            g.load_library(mlp)
            g.wait_ge(s_idx, 16)
            for q, (s, t) in _piece_order():
                n = STREAM_N[s]
                c0, _ = lay[(s, t)]
                dst, k = g_dst(s, t)
                g.dma_gather(
                    dst.rearrange("p (k d) -> p k d", k=k),
                    src_of[s][:],
                    idx_t[:, c0 : c0 + n // 16],
                    n,
                    n,
                    VEC,
                    single_packet=False,
                    queue_num=q,
                ).then_inc(s_tile[t], 16)

        @block.vector
        def _(v: bass.BassVectorEngine):
            npieces = {t: sum(1 for q in PIECES.values() for (s, tt) in q if tt == t) for t in range(TILES)}

            def gsl(t, a, b):
                base = t * G_SLOTS * VEC
                return G[:, base + a * VEC : base + b * VEC]

            for t in range(TILES):
                v.wait_ge(s_tile[t], npieces[t] * 16)
                # ctx sum as a contiguous binary add tree: (A+B) 8 -> 4 -> 2 -> 1, + doc
                v.tensor_tensor(out=s8[:, :], in0=gsl(t, 0, 8), in1=gsl(t, 8, 16), op=mybir.AluOpType.add)
                v.drain()
                v.tensor_tensor(out=s4[:, :], in0=s8[:, : 4 * VEC], in1=s8[:, 4 * VEC :], op=mybir.AluOpType.add)
                v.drain()
                v.tensor_tensor(out=s2[:, :], in0=s4[:, : 2 * VEC], in1=s4[:, 2 * VEC :], op=mybir.AluOpType.add)
                v.drain()
                v.tensor_tensor(out=s1[:, :], in0=s2[:, :VEC], in1=s2[:, VEC:], op=mybir.AluOpType.add)
                v.drain()
                v.tensor_tensor(
                    out=xb[:, t * VEC : (t + 1) * VEC],
                    in0=s1[:, :],
                    in1=gsl(t, 16, 17),
                    op=mybir.AluOpType.add,
                )
                v.drain()
                # prod[p, j, d] = xb[p, d] * NB[p, t, j, d], j in [0, 12)
                v.tensor_tensor(
                    out=prod[:, :].rearrange("p (j d) -> p j d", j=N_SLOTS),
                    in0=xb[:, t * VEC : (t + 1) * VEC][:, None, :].to_broadcast(
                        [P, N_SLOTS, VEC]
                    ),
                    in1=NB[
                        :, t * N_SLOTS * VEC : (t + 1) * N_SLOTS * VEC
                    ].rearrange("p (j d) -> p j d", j=N_SLOTS),
                    op=mybir.AluOpType.mult,
                )
                # per-slot dots over contiguous d, then lo+hi halves
                v.drain()
                v.tensor_reduce(
                    out=sc12[:, :],
                    in_=prod[:, :].rearrange("p (j d) -> p j d", j=N_SLOTS),
                    axis=mybir.AxisListType.X,
                    op=mybir.AluOpType.add,
                )
                v.drain()
                v.tensor_tensor(
                    out=sc[:, t * N_NOISE : (t + 1) * N_NOISE],
                    in0=sc12[:, :N_NOISE],
                    in1=sc12[:, N_NOISE:],
                    op=mybir.AluOpType.add,
                )
                v.drain().then_inc(s_vec, 1)

    nc.compile()
    return nc


def get_nc():
    global _nc_cache
    if _nc_cache is None:
        _nc_cache = build_nc()
    return _nc_cache


def _wrap16(vals):
    """Index list -> [128, n/16] int16 (wrapped in 16 partitions, replicated
    to all 8 Q7 core-pair groups)."""
    n = len(vals)
    assert n % 16 == 0
    blk = np.asarray(vals, dtype=np.int16).reshape(-1, 16).T  # [16, n/16]
    return np.tile(blk, (8, 1))  # [128, n/16]


def make_host_inputs(context_ids, doc_ids, target_noise_ids, D, W, O):
    bf16 = ml_dtypes.bfloat16
    doc_ids = np.asarray(doc_ids, dtype=np.int64)
    ctx = np.asarray(context_ids, dtype=np.int64)
    noi = np.asarray(target_noise_ids, dtype=np.int64)

    zrow = np.zeros((1, VEC), dtype=bf16)
    W16 = np.asarray(W, dtype=np.float32).astype(bf16)
    OT16 = np.ascontiguousarray(np.asarray(O, dtype=np.float32).T).astype(bf16)
    D16 = np.asarray(D, dtype=np.float32).astype(bf16)
    wlo = np.concatenate([zrow, W16[:WB]], axis=0)
    whi = np.concatenate([zrow, W16[WB:]], axis=0)
    olo = np.concatenate([zrow, OT16[:WB]], axis=0)
    ohi = np.concatenate([zrow, OT16[WB:]], axis=0)

    # sort items by doc id so each core's doc window fits 32768 rows
    perm = np.argsort(doc_ids, kind="stable")
    lay = _idx_layout()

    in_maps = []
    bases = []
    for c in range(N_CORES):
        items = perm[c * BPC : (c + 1) * BPC]  # 512 global item ids
        d_ids = doc_ids[items]
        base = int(d_ids.min())
        span = int(d_ids.max()) - base
        assert span < DTAB_R, f"core {c} doc span {span} >= {DTAB_R}"
        dtab = D16[base : base + DTAB_R]
        if dtab.shape[0] < DTAB_R:
            dtab = np.concatenate(
                [dtab, np.zeros((DTAB_R - dtab.shape[0], VEC), dtype=bf16)], axis=0
            )
        bases.append(base)

        # per-(stream, tile) index lists; position i = col*128 + p ->
        # (item tile*128+p, slot col)
        idx_arr = np.empty((P, IDX_COLS), dtype=np.int16)
        c_ids = ctx[items]  # [512, 8]
        n_ids = noi[items]  # [512, 6]
        c_lo = np.where(c_ids < WB, c_ids + 1, 0).astype(np.int16)
        c_hi = np.where(c_ids >= WB, c_ids - WB + 1, 0).astype(np.int16)
        n_lo = np.where(n_ids < WB, n_ids + 1, 0).astype(np.int16)
        n_hi = np.where(n_ids >= WB, n_ids - WB + 1, 0).astype(np.int16)
        d_loc = (d_ids - base).astype(np.int16)  # [512]
        for (s, t), (c0, n) in lay.items():
            rows = slice(t * P, (t + 1) * P)
            if s == "doc":
                vals = d_loc[rows]  # [128] -> position p
            else:
                src = {"ctxA": c_lo, "ctxB": c_hi, "noiA": n_lo, "noiB": n_hi}[s]
                # [128 items, k slots] -> positions col*128 + p
                vals = src[rows].T.reshape(-1)
            idx_arr[:, c0 : c0 + n // 16] = _wrap16(vals)

        in_maps.append(
            {
                "wlo": wlo,
                "whi": whi,
                "olo": olo,
                "ohi": ohi,
                "dtab": dtab,
                "idx": np.ascontiguousarray(idx_arr),
            }
        )
    return in_maps, perm


def unshard_output(outs, perm):
    """outs: 8x [128, 24] f32 -> scores [4096, 6] in original item order."""
    scores_sorted = np.concatenate(
        [
            np.asarray(o, dtype=np.float32).reshape(P, TILES, N_NOISE).transpose(1, 0, 2).reshape(BPC, N_NOISE)
            for o in outs
        ],
        axis=0,
    )
    scores = np.empty_like(scores_sorted)
    scores[perm] = scores_sorted
    return scores


def _install_profile_hook():
    import types

    if "antenv.axon_hooks" in sys.modules:
        return
    import antenv
    from trn_agent_boot.trn_boot import _ntff_profile_via_ctypes

    mod = types.ModuleType("antenv.axon_hooks")
    _state = {"hook": _ntff_profile_via_ctypes("/opt/axon/libaxon_pjrt.so")}
    mod.set_axon_ntff_profile_hook = lambda h: _state.__setitem__("hook", h)
    mod.get_axon_ntff_profile_hook = lambda: _state["hook"]
    sys.modules["antenv.axon_hooks"] = mod
    antenv.axon_hooks = mod


def kernel(context_ids, doc_ids, target_noise_ids, D, W, O, _trace=False):
    if _trace:
        _install_profile_hook()
    nc = get_nc()
    in_maps, perm = make_host_inputs(context_ids, doc_ids, target_noise_ids, D, W, O)
    res = run_bass_kernel_spmd(
        nc, in_maps, core_ids=list(range(N_CORES)), trace=_trace
    )
    scores = unshard_output([res.results[c]["out"] for c in range(N_CORES)], perm)
    if _trace:
        kernel.last_exec_time_ns = res.exec_time_ns
        kernel.last_results = res
    return scores


# revision 3
# speedup vs baseline: 1.0290x; 1.0290x over previous
"""Doc2vec-style embedding lookup + negative-sampling scores on 8 trn2 cores.

reference:
    x[b, :] = D[doc_ids[b]] + sum_c W[context_ids[b, c]]      # (B, 256)
    scores[b, k] = dot(x[b], O[:, target_noise_ids[b, k]])    # (B, 6)

Strategy: data-parallel over batch (512 items/core), tables replicated.
Host concatenates [D; W; O.T] into one row table so every lookup is a row
gather from a single DRAM tensor; each batch item needs 15 rows
(1 doc + 8 ctx + 6 noise).  Per core: 4 batch-tiles of 128 items; each tile
is ONE indirect DMA gathering 128x15 rows into SBUF, then a DVE strided
tensor_reduce sums the 9 embedding rows into x, and 6 fused
tensor_tensor_reduce ops produce the dot-product scores.
"""

import sys

sys.path.insert(0, "/opt/trn_rl_repo")

from contextlib import ExitStack

import numpy as np

from concourse import bacc, bass, mybir, tile
from concourse.bass_utils import run_bass_kernel_spmd

VEC = 256
N_DOCS = 100000
N_WORDS = 50000
B = 4096
N_CTX = 8
N_NOISE = 6
N_CORES = 8
BPC = B // N_CORES  # 512 batch items per core
P = 128
TILES = BPC // P  # 4 batch tiles per core
RPI = 1 + N_CTX + N_NOISE  # 15 gathered rows per batch item
T_ROWS = N_DOCS + 2 * N_WORDS  # 200000

_nc_cache = None


def build_nc_raw():
    """Raw-Bass (no TileContext) pipeline: avoids Tile's ~7us preamble EVSEM
    butterfly, per-gather sem bookkeeping (~310ns/gather), and the end
    barrier.  Sync: per-batch-tile semaphores with exact counts (16 incs per
    DMA x 9 or 6 DMAs), so a sem reaching its target proves every SDMA engine
    finished that tile's descriptors."""
    nc = bass.Bass(target_bir_lowering=False, debug=False, num_swdge_queues=2)
    tbl = nc.declare_dram_parameter(
        "tbl", [T_ROWS, VEC], mybir.dt.float32, isOutput=False
    )
    idx = nc.declare_dram_parameter(
        "idx", [P, TILES * RPI], mybir.dt.int32, isOutput=False
    )
    out = nc.declare_dram_parameter(
        "out", [P, TILES * N_NOISE], mybir.dt.float32, isOutput=True
    )

    with ExitStack() as ctx:
        block = ctx.enter_context(nc.Block(no_gpsimd_drain=True))
        sem_idx = ctx.enter_context(nc.semaphore("sem_idx"))
        sem_x = [ctx.enter_context(nc.semaphore(f"sem_x{j}")) for j in range(TILES)]
        sem_n = [
            [
                ctx.enter_context(nc.semaphore(f"sem_n{j}_{h}"))
                for h in range(2)
            ]
            for j in range(TILES)
        ]
        sem_vec = ctx.enter_context(nc.semaphore("sem_vec"))
        sem_out = ctx.enter_context(nc.semaphore("sem_out"))
        idx_t = ctx.enter_context(
            nc.sbuf_tensor("idx_t", [P, TILES * RPI], mybir.dt.int32)
        )
        gbuf = ctx.enter_context(
            nc.sbuf_tensor("gbuf", [P, TILES * RPI * VEC], mybir.dt.float32)
        )
        x4 = ctx.enter_context(nc.sbuf_tensor("x4", [P, TILES * VEC], mybir.dt.float32))
        prod6 = ctx.enter_context(
            nc.sbuf_tensor("prod6", [P, N_NOISE * VEC], mybir.dt.float32)
        )
        score = ctx.enter_context(
            nc.sbuf_tensor("score", [P, TILES * N_NOISE], mybir.dt.float32)
        )

        @block.sync
        def _(s: bass.BassEngine):
            s.dma_start(out=idx_t[:, :], in_=idx[:, :]).then_inc(sem_idx, 16)
            s.wait_ge(sem_vec, 1)
            s.dma_start(out=out[:, :], in_=score[:, :]).then_inc(sem_out, 16)
            s.wait_ge(sem_out, 16)

        @block.gpsimd
        def _(g: bass.BassGpSimd):
            g.wait_ge(sem_idx, 16)
            for j in range(TILES):
                for r in range(RPI):
                    col = j * RPI + r
                    ins = g.indirect_dma_start(
                        out=gbuf[:, col * VEC : (col + 1) * VEC],
                        out_offset=None,
                        in_=tbl[:],
                        in_offset=bass.IndirectOffsetOnAxis(
                            ap=idx_t[:, col : col + 1], axis=0
                        ),
                    )
                    if r <= N_CTX:
                        tgt = sem_x[j]
                    else:
                        tgt = sem_n[j][(r - 1 - N_CTX) // (N_NOISE // 2)]
                    ins.then_inc(tgt, 16)
                    if col % 2 == 1:
                        ins.queue = "qPoolDynamic1"

        @block.vector
        def _(v: bass.BassVectorEngine):
            for j in range(TILES):
                v.wait_ge(sem_x[j], (1 + N_CTX) * 16)
                v.tensor_reduce(
                    out=x4[:, j * VEC : (j + 1) * VEC],
                    in_=gbuf[
                        :, j * RPI * VEC : (j * RPI + 1 + N_CTX) * VEC
                    ].rearrange("p (r d) -> p d r", r=1 + N_CTX),
                    axis=mybir.AxisListType.X,
                    op=mybir.AluOpType.add,
                )
                v.drain()  # retire x4 write before tt reads it
                # noise scores in two 3-slot chunks so the last chunk's DVE
                # work after the final gather is half-sized
                half = N_NOISE // 2
                for h in range(2):
                    k0 = h * half
                    v.wait_ge(sem_n[j][h], half * 16)
                    pslice = prod6[:, k0 * VEC : (k0 + half) * VEC]
                    v.tensor_tensor(
                        out=pslice.rearrange("p (k d) -> p k d", k=half),
                        in0=x4[:, j * VEC : (j + 1) * VEC][:, None, :].to_broadcast(
                            [P, half, VEC]
                        ),
                        in1=gbuf[
                            :,
                            (j * RPI + 1 + N_CTX + k0) * VEC : (
                                j * RPI + 1 + N_CTX + k0 + half
                            )
                            * VEC,
                        ].rearrange("p (k d) -> p k d", k=half),
                        op=mybir.AluOpType.mult,
                    )
                    v.drain()  # retire prod6 chunk before reduce reads it
                    v.tensor_reduce(
                        out=score[:, j * N_NOISE + k0 : j * N_NOISE + k0 + half],
                        in_=pslice.rearrange("p (k d) -> p k d", k=half),
                        axis=mybir.AxisListType.X,
                        op=mybir.AluOpType.add,
                    )
                v.drain()  # retire score before next-j reuse / final store
            v.drain().then_inc(sem_vec, 1)

    return nc


def build_nc():
    nc = bacc.Bacc(None, target_bir_lowering=False, debug=True)
    tbl = nc.declare_dram_parameter(
        "tbl", [T_ROWS, VEC], mybir.dt.float32, isOutput=False
    )
    idx = nc.declare_dram_parameter(
        "idx", [P, TILES * RPI], mybir.dt.int32, isOutput=False
    )
    out = nc.declare_dram_parameter(
        "out", [P, TILES * N_NOISE], mybir.dt.float32, isOutput=True
    )

    with tile.TileContext(nc) as tc:
        with (
            tc.tile_pool(name="gpool", bufs=TILES) as gpool,
            tc.tile_pool(name="vpool", bufs=2) as vpool,
            tc.tile_pool(name="cpool", bufs=1) as cpool,
        ):
            idx_t = cpool.tile([P, TILES * RPI], mybir.dt.int32)
            nc.sync.dma_start(out=idx_t[:], in_=idx[:])
            score_t = cpool.tile([P, TILES * N_NOISE], mybir.dt.float32)
            for j in range(TILES):
                g = gpool.tile([P, RPI * VEC], mybir.dt.float32, tag="g")
                # HW indirect DMA uses ONE index per partition (the rest of
                # the offset AP's free dim is ignored and the descriptor just
                # reads contiguous bytes), so emit one gather per row-slot.
                for r in range(RPI):
                    col = j * RPI + r
                    nc.gpsimd.indirect_dma_start(
                        out=g[:, r * VEC : (r + 1) * VEC],
                        out_offset=None,
                        in_=tbl[:],
                        in_offset=bass.IndirectOffsetOnAxis(
                            ap=idx_t[:, col : col + 1], axis=0
                        ),
                    )
                x = vpool.tile([P, VEC], mybir.dt.float32, tag="x")
                # x[p, d] = sum_r g[p, r*VEC + d] over the 9 embedding rows
                nc.vector.tensor_reduce(
                    out=x[:],
                    in_=g[:, : (1 + N_CTX) * VEC].rearrange(
                        "p (r d) -> p d r", r=1 + N_CTX
                    ),
                    axis=mybir.AxisListType.X,
                    op=mybir.AluOpType.add,
                )
                # scores for all 6 noise slots at once:
                # prod6[p, k, d] = x[p, d] * g[p, (9+k)*VEC + d]; reduce over d
                prod6 = vpool.tile([P, N_NOISE * VEC], mybir.dt.float32, tag="prod6")
                nc.vector.tensor_tensor(
                    out=prod6[:].rearrange("p (k d) -> p k d", k=N_NOISE),
                    in0=x[:, None, :].to_broadcast([P, N_NOISE, VEC]),
                    in1=g[:, (1 + N_CTX) * VEC : RPI * VEC].rearrange(
                        "p (k d) -> p k d", k=N_NOISE
                    ),
                    op=mybir.AluOpType.mult,
                )
                nc.vector.tensor_reduce(
                    out=score_t[:, j * N_NOISE : (j + 1) * N_NOISE],
                    in_=prod6[:].rearrange("p (k d) -> p k d", k=N_NOISE),
                    axis=mybir.AxisListType.X,
                    op=mybir.AluOpType.add,
                )
            nc.sync.dma_start(out=out[:], in_=score_t[:])
    nc.compile()
    return nc


def get_nc():
    global _nc_cache
    if _nc_cache is None:
        _nc_cache = build_nc_raw()
    return _nc_cache


def make_host_inputs(context_ids, doc_ids, target_noise_ids, D, W, O):
    """Returns (tbl [200000,256] f32, per-core idx tiles [8][128, 60] i32)."""
    tbl = np.concatenate(
        [
            np.asarray(D, dtype=np.float32),
            np.asarray(W, dtype=np.float32),
            np.ascontiguousarray(np.asarray(O, dtype=np.float32).T),
        ],
        axis=0,
    )
    doc = np.asarray(doc_ids, dtype=np.int64).reshape(B, 1)
    ctx = np.asarray(context_ids, dtype=np.int64) + N_DOCS
    noi = np.asarray(target_noise_ids, dtype=np.int64) + (N_DOCS + N_WORDS)
    rows = np.concatenate([doc, ctx, noi], axis=1).astype(np.int32)  # [B, 15]
    idx_cores = []
    for c in range(N_CORES):
        r = rows[c * BPC : (c + 1) * BPC]  # [512, 15]
        idx_cores.append(
            np.ascontiguousarray(
                r.reshape(TILES, P, RPI).transpose(1, 0, 2).reshape(P, TILES * RPI)
            )
        )
    return tbl, idx_cores


def unshard_output(outs):
    """outs: list of 8 arrays [128, 24] -> scores [4096, 6] f32."""
    parts = []
    for o in outs:
        parts.append(
            np.ascontiguousarray(
                np.asarray(o, dtype=np.float32)
                .reshape(P, TILES, N_NOISE)
                .transpose(1, 0, 2)
                .reshape(BPC, N_NOISE)
            )
        )
    return np.concatenate(parts, axis=0)


def _install_profile_hook():
    """The agent image lacks ``antenv.axon_hooks``; inject the 3-line shim so
    run_bass_kernel_spmd(trace=True) can find the NTFF hook (the actual
    profiling impl lives in trn_agent_boot.trn_boot)."""
    import types

    if "antenv.axon_hooks" in sys.modules:
        return
    import antenv
    from trn_agent_boot.trn_boot import _ntff_profile_via_ctypes

    mod = types.ModuleType("antenv.axon_hooks")
    _state = {"hook": _ntff_profile_via_ctypes("/opt/axon/libaxon_pjrt.so")}
    mod.set_axon_ntff_profile_hook = lambda h: _state.__setitem__("hook", h)
    mod.get_axon_ntff_profile_hook = lambda: _state["hook"]
    sys.modules["antenv.axon_hooks"] = mod
    antenv.axon_hooks = mod


def kernel(context_ids, doc_ids, target_noise_ids, D, W, O, _trace=False):
    if _trace:
        _install_profile_hook()
    nc = get_nc()
    tbl, idx_cores = make_host_inputs(
        context_ids, doc_ids, target_noise_ids, D, W, O
    )
    in_maps = [{"tbl": tbl, "idx": idx_cores[c]} for c in range(N_CORES)]
    res = run_bass_kernel_spmd(
        nc, in_maps, core_ids=list(range(N_CORES)), trace=_trace
    )
    scores = unshard_output([res.results[c]["out"] for c in range(N_CORES)])
    if _trace:
        kernel.last_exec_time_ns = res.exec_time_ns
        kernel.last_results = res
    return scores

